# revision 19
# baseline (speedup 1.0000x reference)
"""GCN block (3 layers) on 8 trn2 NeuronCores, data-parallel over batch.

Math: each layer is X' = (adj + I) @ leaky_relu(X @ W).
Fold each layer's weight into the previous layer's output (associativity:
(A @ H) @ W == A @ (H @ W)) so every layer is one big matmul against adj:

    H0 = lrelu(X0 W0)              (host-side input prep, fp16)
    G_l = H_l W_{l+1}              (tiny matmul, W3 = I)
    Z_{l+1} = adj @ G_l + G_l  ;   H_{l+1} = lrelu(Z_{l+1})

Scaling invariant: SBUF holds ht' = S*H (fp16, S=1024 keeps |S*Z| well
under fp16 max).  adj^T is pre-scaled by S and quantized to fp8e4 (adj
~ U[0, 2/N] ~ 5e-4 is below e4m3's subnormal floor unscaled).  Then ONE
weight slot W serves both per-layer matmuls:

    tiny:  psum_g = ht' @ W = S*G      -> cast*(1/S) -> G_q (fp8)
    fold:  psum  += W^T_blk @ ht'^T    =  S*G^T      (exact identity path)
    big:   psum  += A_s @ G_q          =  S*(adj@G)
    leaky: ht'_next = NEG*psum + (1-NEG)*relu(psum) = S*lrelu(Z)
           (ACT relu + DVE STT, positive homogeneity of leaky_relu;
           native Lrelu's alpha parameter is ignored by the HW)

A_s (16 MiB fp8) is DMA'd once (16 contiguous 1-MiB panels, layer-0
matmuls chase the arrivals) and stays SBUF-resident for all three layers:
HBM traffic is ~18 MB total.  Big matmuls run fp8 DoubleRow (2 k-tiles of
128 per instruction, ~216 ns per 512-wide pair on HW).

Per core: 8 samples x 16 features = 128 = partition width.  Layouts:
    T-layout  [c=(b,d), m]   (128 partitions, N free)
    N-layout  [m, c]         (m partitions, 128 free)

Layer-0 big phase runs pair-outer chasing panel DMAs, with pair 0
deferred as filler for the last panel's ~2us completion-receipt window
(keeps PE busy and avoids the p-state drop), and each chunk's epilogue
emitted inline right after its stop matmul so chunks retire staggered.
Layers 1-2 run chunk-outer so each psum chunk retires early and its
epilogue overlaps the remaining matmuls.  Epilogue ops alternate the
ACT/DVE engines so the post-window chains run in parallel.
"""

import numpy as np

N_FULL = 4096
D = 16
B_FULL = 64
NCORES = 8
B_CORE = B_FULL // NCORES  # 8
C = B_CORE * D  # 128 partitions
P = 128
FREE = 512
NCH = N_FULL // FREE   # 8 psum chunks
NT = N_FULL // P       # 32 m-tiles
NPAIR = NT // 2        # 16 DoubleRow pairs / A panels
NEG_SLOPE = 0.2
S_ADJ = 1024.0

_CACHE = {}


def _build_nc():
    import concourse.mybir as mybir
    import concourse.tile as tile
    from concourse import bacc

    f32 = mybir.dt.float32
    f16 = mybir.dt.float16
    f8 = mybir.dt.float8e4
    DR = mybir.MatmulPerfMode.DoubleRow
    MULT = mybir.AluOpType.mult
    MAX = mybir.AluOpType.max
    COPY = mybir.ActivationFunctionType.Copy
    n = N_FULL

    nc = bacc.Bacc(
        "TRN2", target_bir_lowering=False, debug=False, num_devices=NCORES
    )
    # S*h0^T with the 3 weight blocks (W1, W2, I block-diag) appended
    # column-wise: one contiguous DMA, one trigger behind panel 0
    h0_h = nc.dram_tensor("h0", [C, n + 3 * P], f16, kind="ExternalInput")
    # A panels: row i holds pair i, free layout [t*n + j] per partition
    # (t = k-subtile 0/1); 1 MiB per panel DMA = max-bandwidth regime
    at_h = nc.dram_tensor("at", [NPAIR, P, 2 * n], f8, kind="ExternalInput")
    # fp16 output: halves the store traffic; 5e-4 rounding ~ noise here
    out_h = nc.dram_tensor("out", [C, n], f16, kind="ExternalOutput")

    def leaky(dest, ps, pool):
        # dest = S*lrelu(ps/S) = 0.2*ps + 0.8*relu(ps) (homogeneity);
        # ACT + DVE in parallel, each reading PSUM once.
        t = pool.tile([P, FREE], f16, tag="lk")
        nc.scalar.activation(
            t[:], ps[:], mybir.ActivationFunctionType.Relu,
            scale=1.0 - NEG_SLOPE,
        )
        nc.vector.scalar_tensor_tensor(
            dest, ps[:], NEG_SLOPE, t[:],
            mybir.AluOpType.mult, mybir.AluOpType.add,
        )

    with tile.TileContext(nc) as tc:
        with (
            tc.tile_pool(name="const", bufs=1) as constp,
            tc.tile_pool(name="ht", bufs=2) as htp,
            tc.tile_pool(name="g3", bufs=2) as g3p,
            tc.tile_pool(name="outp", bufs=4) as outp,
            tc.tile_pool(name="lk", bufs=4) as lkp,
            tc.tile_pool(name="ps", bufs=8, space="PSUM") as psp,
        ):
            hw_sb = constp.tile([C, n + 3 * P], f16)
            ht_cur = hw_sb

            def wv(idx):
                return hw_sb[:, n + idx * P:n + (idx + 1) * P]
            # resident scaled-adj^T panels, [128, 2, 4096] fp8 each
            at3 = [
                constp.tile([P, 2, n], f8, name=f"at{i}") for i in range(NPAIR)
            ]

            def panel_dma(i):
                nc.sync.dma_start(
                    at3[i][:],
                    at_h[i, :, :].rearrange("p (t j) -> p t j", t=2),
                )
            # panel 0 first so the A stream starts one trigger earlier;
            # h0w slots in behind it (layer-0 PE has ample slack)
            panel_dma(0)
            nc.sync.dma_start(hw_sb[:], h0_h[:])
            for i in range(1, NPAIR):
                panel_dma(i)

            def rhs_ap(t, ch):
                # A_s^T pair t, output chunk ch -> [128, 2, 512] fp8 AP
                return at3[t][:, :, ch * FREE:(ch + 1) * FREE]

            for layer in range(3):
                widx = layer  # W1, W2, I: shared by tiny and fold
                last = layer == 2

                # tiny: S*G[m,c] = ht'[m,:] @ W, 4 m-tiles packed per psum
                # bank, then one cast (scale 1/S) per bank -> fp8 G;
                # casts alternate DVE/ACT: two parallel chains
                g3c = g3p.tile([P, NT, P], f8)

                def tiny_group(q, g3c=g3c, widx=widx):
                    psg = psp.tile([P, 4, P], f32, tag="ps")
                    for j in range(4):
                        mt = 4 * q + j
                        nc.tensor.matmul(
                            psg[:, j, :],
                            ht_cur[:, mt * P:(mt + 1) * P],
                            wv(widx),
                            start=(j == 0), stop=(j == 3),
                            skip_group_check=True,
                        )
                    if q % 2 == 0:
                        nc.vector.tensor_scalar_mul(
                            g3c[:, 4 * q:4 * q + 4, :], psg[:], 1.0 / S_ADJ,
                        )
                    else:
                        nc.scalar.activation(
                            g3c[:, 4 * q:4 * q + 4, :], psg[:], COPY,
                            scale=1.0 / S_ADJ,
                        )

                def fold_mm(ps, ch):
                    nc.tensor.matmul(
                        ps[:],
                        wv(widx),
                        ht_cur[:, ch * FREE:(ch + 1) * FREE],
                        start=True, stop=False,
                        skip_group_check=True,
                    )

                def dr_mm(ps, t, ch, stop=False):
                    nc.tensor.matmul(
                        ps[:],
                        g3c[:, 2 * t:2 * t + 2, :],
                        rhs_ap(t, ch),
                        start=False, stop=stop,
                        perf_mode=DR,
                        skip_group_check=True,
                    )

                def epilogue(ch, ps, ht_next):
                    if not last:
                        leaky(ht_next[:, ch * FREE:(ch + 1) * FREE], ps, lkp)
                        return
                    oc = outp.tile([C, FREE], f16, tag="oc")
                    half = FREE // 2
                    if ch == NCH - 1:
                        # final chunk: both halves on DVE (ACT can lag
                        # here), half-DMAs so the store overlaps the
                        # trailing copy
                        nc.vector.tensor_scalar_mul(
                            oc[:, :half], ps[:, :half], 1.0 / S_ADJ
                        )
                        nc.sync.dma_start(
                            out_h[:, ch * FREE:ch * FREE + half],
                            oc[:, :half],
                        )
                        nc.vector.tensor_scalar_mul(
                            oc[:, half:], ps[:, half:], 1.0 / S_ADJ
                        )
                        nc.sync.dma_start(
                            out_h[:, ch * FREE + half:(ch + 1) * FREE],
                            oc[:, half:],
                        )
                    else:
                        # halves on DVE + ACT concurrently, one DMA
                        nc.vector.tensor_scalar_mul(
                            oc[:, :half], ps[:, :half], 1.0 / S_ADJ
                        )
                        nc.scalar.activation(
                            oc[:, half:], ps[:, half:], COPY,
                            scale=1.0 / S_ADJ,
                        )
                        nc.sync.dma_start(
                            out_h[:, ch * FREE:(ch + 1) * FREE], oc[:]
                        )

                # big: psum[c, n] = W^T_blk @ ht'^T  (identity fold, fp16)
                #                 + sum_pairs G_q^T A_s^T (fp8 DoubleRow)
                if layer == 0:
                    for q in range(NT // 4):
                        tiny_group(q)
                    # pair-outer: chase the panel DMAs.  Pair 0 deferred
                    # as filler for the last panel's receipt window; stop
                    # on pair 15 with the epilogue emitted inline so
                    # chunks retire staggered.
                    ps_list = [
                        psp.tile([P, FREE], f32, tag="ps", name=f"psc{i}")
                        for i in range(NCH)
                    ]
                    for ch in range(NCH):
                        fold_mm(ps_list[ch], ch)
                    for t in range(1, NPAIR - 1):
                        for ch in range(NCH):
                            dr_mm(ps_list[ch], t, ch)
                    # filler: pair 0 (resident since ~t=12us)
                    for ch in range(NCH):
                        dr_mm(ps_list[ch], 0, ch)
                    ht_next = htp.tile([C, n], f16, name="htn")
                    for ch in range(NCH):
                        dr_mm(ps_list[ch], NPAIR - 1, ch, stop=True)
                        leaky(ht_next[:, ch * FREE:(ch + 1) * FREE],
                              ps_list[ch], lkp)
                    ht_cur = ht_next
                else:
                    # chunk-outer, with bracket 0's fold + DR pairs
                    # interleaved into the tiny phase so the PE consumes
                    # each cast as it lands instead of stalling after
                    ht_next = None if last else htp.tile([C, n], f16,
                                                         name="htn")
                    interleave = False
                    if interleave:
                        tiny_group(0)
                        ps0 = psp.tile([P, FREE], f32, tag="ps", name="psc0")
                        fold_mm(ps0, 0)
                        for q in range(1, NT // 4):
                            tiny_group(q)
                            dr_mm(ps0, 2 * (q - 1), 0)
                            dr_mm(ps0, 2 * (q - 1) + 1, 0)
                        dr_mm(ps0, NPAIR - 2, 0)
                        dr_mm(ps0, NPAIR - 1, 0, stop=True)
                    else:
                        for q in range(NT // 4):
                            tiny_group(q)
                        ps0 = psp.tile([P, FREE], f32, tag="ps", name="psc0")
                        fold_mm(ps0, 0)
                        for t in range(NPAIR):
                            dr_mm(ps0, t, 0, stop=(t == NPAIR - 1))
                    epilogue(0, ps0, ht_next)
                    for ch in range(1, NCH):
                        ps = psp.tile([P, FREE], f32, tag="ps",
                                      name=f"psc{ch}")
                        fold_mm(ps, ch)
                        for t in range(NPAIR):
                            dr_mm(ps, t, ch, stop=(t == NPAIR - 1))
                        epilogue(ch, ps, ht_next)
                    ht_cur = ht_next

    nc.compile()
    return nc


def _get_nc():
    if "nc" not in _CACHE:
        _CACHE["nc"] = _build_nc()
    return _CACHE["nc"]


def _block_diag(w, reps):
    d = w.shape[0]
    out = np.zeros((reps * d, reps * d), dtype=np.float32)
    for b in range(reps):
        out[b * d:(b + 1) * d, b * d:(b + 1) * d] = w
    return out


def prepare_inputs(x, adj, Identity, W0, W1, W2):
    import concourse.mybir as mybir

    np_f8 = mybir.dt.np(mybir.dt.float8e4)
    n = N_FULL
    reps = C // D

    # adj with any deviation of Identity from eye folded in (Identity is
    # eye in the reference; the subtraction is exact in that case)
    a_eff = np.asarray(adj, np.float32) + np.asarray(Identity, np.float32) \
        - np.eye(n, dtype=np.float32)
    at_q = (np.ascontiguousarray(a_eff.T) * S_ADJ).astype(np_f8)
    # [m, col] -> panel rows [i, p, t*n + j], m = 256 i + 128 t + p
    ap = at_q.reshape(NPAIR, 2, P, n)                 # [i, t, p, j]
    at_prep = np.ascontiguousarray(
        ap.transpose(0, 2, 1, 3)                      # [i, p, t, j]
    ).reshape(NPAIR, P, 2 * n)

    wb = [_block_diag(np.asarray(w, np.float32), reps) for w in (W1, W2)]
    eye = np.eye(C, dtype=np.float32)
    w_all = np.stack([wb[0], wb[1], eye]).astype(np.float16)
    # per-partition layout [p, w*128] for the columns appended to h0
    w_flat = np.ascontiguousarray(w_all.transpose(1, 0, 2)).reshape(C, 3 * P)

    # host-side H0 = lrelu(x @ W0), shipped transposed, S-scaled, fp16
    xw0 = np.einsum("bnd,de->bne", np.asarray(x, np.float32),
                    np.asarray(W0, np.float32))
    h0 = np.where(xw0 > 0, xw0, NEG_SLOPE * xw0) * S_ADJ
    in_maps = []
    for core in range(NCORES):
        hs = h0[core * B_CORE:(core + 1) * B_CORE]      # (B_CORE, n, D)
        h0t = np.ascontiguousarray(
            hs.transpose(0, 2, 1).reshape(C, n)
        ).astype(np.float16)
        h0w = np.ascontiguousarray(np.hstack([h0t, w_flat]))
        in_maps.append({"h0": h0w, "at": at_prep})
    return in_maps


def gather_output(results, b_full=B_FULL):
    out = np.empty((b_full, N_FULL, D), dtype=np.float32)
    for core in range(NCORES):
        oc = np.asarray(results[core]["out"]).astype(np.float32).reshape(
            B_CORE, D, N_FULL)
        out[core * B_CORE:(core + 1) * B_CORE] = oc.transpose(0, 2, 1)
    return out


def run(x, adj, Identity, W0, W1, W2, trace=False, **_ignored):
    from concourse.bass_utils import run_bass_kernel_spmd

    nc = _get_nc()
    in_maps = prepare_inputs(x, adj, Identity, W0, W1, W2)
    res = run_bass_kernel_spmd(nc, in_maps, list(range(NCORES)), trace=trace)
    out = gather_output(res.results, x.shape[0])
    return out, res


def kernel(x, adj, Identity, W0, W1, W2):
    out, _ = run(x, adj, Identity, W0, W1, W2)
    return out
